# revision 1
# baseline (speedup 1.0000x reference)
"""BinarizeConv2dSDP kernel for Trainium2 (8 NeuronCores, data-parallel over batch).

out = conv2d(sign(x), sign(M + sum_k rv[k] * Z[k]), stride 1, pad 1) * Alpha

The reference's rsqrt pre-normalization is strictly positive and multiplicative,
so sign(w) is unaffected: binary weights are sign(M + rv@Z).

Measured 87.5-90.3us on hardware (v1 baseline: 97.2us same-session), rel err
2.1e-4; the conv phase runs with ZERO stalls end to end.  Run-to-run device
variance is +-2-10% on this shared part.

Strategy (from v1's trace analysis; per-core traffic/PE floors drove each):
  - fp16 I/O on the device: x is uploaded as fp16 (sign(fp16(x)) == sign(x)
    for all practically occurring values) and out is stored as fp16 (conv
    counts <= 1152 are fp16-exact; only the Alpha multiply rounds, ~2e-4 rel
    err).  This halves the dominant HBM traffic: 29.2MB -> 16.4MB per core
    against the ~435 GB/s aggregate DMA cap, making the kernel PE-bound.
  - p-outer conv loop: each of the 5 DoubleRowSwInterleave weight pairs
    sweeps all 7 row-chunks of an image into 7 PSUM banks, so consecutive
    matmuls share the stationary weights.  tile_legalize still emits a
    standalone LDWEIGHTS per matmul (~236ns cadence, LDWEIGHTS-bound);
    _excise_redundant_ldweights removes the 240 same-weights reloads from
    the scheduled IR (walrus then emits no load for the ldweights=False
    matmuls), dropping the cadence to the ~200ns fp8 peak.  The PE matmul
    order is pinned with nosync deps, else the tile scheduler interleaves
    passes and breaks the same-weights runs.
  - fp8 DoubleRow processes 2 moving taps per column-cycle (the 2x is in the
    K dim): 5 passes x 464 cols x 56 chunks ~= 54us is this formulation's PE
    floor; the conv phase runs gapless at ~63us including pass-leader
    LDWEIGHTS and image-boundary PSUM waits.
  - PSUM->SBUF evictions (with per-channel Alpha scale) split across DVE
    (chunks 0,2,4,6) and ScalarE (1,3,5); GpSimd has no PSUM port, so it
    only carries pad-border memsets, the identity, and SWDGE output DMAs.
    Evictions and output DMAs are flat 2D copies in the padded-row layout
    (the 3D strided PSUM read costs ScalarE 1.6x, and a strided output DMA
    is descriptor-bound, 150us total); the host strips the 2 garbage cols
    per row during the gather, for +3.5% output bytes.
  - Remaining known floors: ~5us of PE p-state ramp during image 0 (the
    387ns-gap train with no semaphore waits).  Pre-warming the PE with
    dummy matmuls does NOT pay: the dummies themselves execute at the
    0.65GHz low state (712ns each) and the ramp progresses far slower
    than the cost model's 3us — 14 dummies cost 10us to save 5.  Also
    ~0.8us x 7 image-boundary stalls and the ~2.5us drain barrier.
  - prologue: Z rides as fp16 (its rv-scaled contribution makes fp16
    rounding harmless — ~0 weight-sign flips, verified bit-identical rel
    err), whole-tensor z DMAs (half-DMAs just doubled the ~650ns SP issue
    cost), wire order z0-z3 / x0 halves / z4 / M / x1-7, and the chain folds
    M LAST so its DMA trails the critical Z wire.  First conv matmul ~20us.
    Rejected alternatives: CC AllGather of rank-sliced weight-gen costs
    ~60us launch overhead (vs ~9us of wire saved); per-pair sign/transpose
    pipelining can't help because the transposes borrow the conv's PSUM
    banks, so the conv must follow all of them anyway.
  - pad-border memsets once per physical ba buffer (3 bufs, manual rotation).
  - outputs: images 0-4 ride SWDGE (never head-of-line blocking input loads
    on the FIFO SP HWDGE ring), 5-6 ride the by-then idle SP ring, and the
    last image drains in four 2-chunk pieces alternating SP/ACT as chunks
    evict.
"""

import numpy as np
from contextlib import ExitStack

import concourse.bass as bass
import concourse.bass_utils as _bass_utils
import concourse.mybir as mybir
import concourse.tile as tile
from concourse.bacc import Bacc
from concourse.bass_utils import run_bass_kernel_spmd

# Rewrite --enable-ldw-opt on the walrus_driver invocation.  Disabled: the
# walrus pass rejects the standalone InstLdweights that tile_legalize emits
# ("InstLdweights is not compatible with LDW optimization"); the redundant
# loads are excised directly in _excise_redundant_ldweights instead.
LDW_OPT = False


def _patch_walrus_ldw_opt():
    if getattr(_bass_utils, "_ldw_opt_patched", False):
        return
    orig = _bass_utils.run_command

    def run_command_ldw(argv, **kwargs):
        if (
            LDW_OPT
            and isinstance(argv, list)
            and any("walrus_driver" in str(a) for a in argv)
        ):
            argv = [
                "--enable-ldw-opt=true" if a == "--enable-ldw-opt=false" else a
                for a in argv
            ]
        return orig(argv, **kwargs)

    _bass_utils.run_command = run_command_ldw
    _bass_utils._ldw_opt_patched = True


_patch_walrus_ldw_opt()

N_CORES = 8
B, C, H, W = 64, 128, 56, 56
BPC = B // N_CORES  # images per core
KS, K = 3, 5
PH, PW = H + 2, W + 2  # zero-padded image
CHUNK_ROWS = 8
N_CHUNKS = H // CHUNK_ROWS
FREE = CHUNK_ROWS * W  # valid output elements per chunk (448)
FREE_R = CHUNK_ROWS * PW  # matmul free dim incl. garbage cols (464 <= 512)
F32 = mybir.dt.float32
F16 = mybir.dt.float16
BF16 = mybir.dt.bfloat16
F8 = mybir.dt.float8e4

# Elide the redundant LDWEIGHTS on matmuls 2..7 of each weight pass (the
# stationary tile is unchanged within a pass).
LDW_ELIDE = True


def _excise_redundant_ldweights(nc):
    """Remove InstLdweights whose stationary tile is already resident.

    tile_legalize pairs every non-f32 InstMatmult with a standalone
    InstLdweights; in the p-outer conv loop 6 of every 7 reload the identical
    weights.  Walrus emits no weight load for an InstMatmult with
    ldweights=False when its standalone load is gone, so the PE array keeps
    the resident weights and the matmul cadence drops from ~236ns
    (LDWEIGHTS-bound) to the raw fp8 DoubleRow rate.  Waits/updates of a
    removed load are merged into the following matmul; Bacc.compile()
    legalizes any wait overflow afterwards.
    """
    import concourse.mybir as _mb

    removed = 0
    for blk in nc.main_func.blocks:
        insts = list(blk.instructions)
        last_sig = None
        keep = []
        for idx, inst in enumerate(insts):
            if isinstance(inst, _mb.InstLdweights):
                a = inst.ins[0]
                sig = (
                    a.memref,
                    a.offset,
                    tuple(tuple(p) for p in a.ap),
                    str(a.dtype),
                )
                nxt = insts[idx + 1] if idx + 1 < len(insts) else None
                if (
                    sig == last_sig
                    and isinstance(nxt, _mb.InstMatmult)
                    and not nxt.is_transpose
                ):
                    si = inst.sync_info
                    if si is not None and (si.on_wait or si.on_update):
                        msi = nxt.sync_info
                        if msi is None:
                            nxt.sync_info = _mb.SyncInfo(
                                on_wait=list(si.on_wait),
                                on_update=list(si.on_update),
                            )
                        else:
                            nxt.sync_info = _mb.SyncInfo(
                                on_wait=list(msi.on_wait) + list(si.on_wait),
                                on_update=list(msi.on_update) + list(si.on_update),
                            )
                    try:
                        nxt.merge_dependencies_from(inst)
                    except Exception:
                        pass
                    removed += 1
                    continue  # drop this reload
                last_sig = sig
            elif isinstance(inst, _mb.InstMatmult):
                if inst.is_transpose:
                    last_sig = None
            keep.append(inst)
        if removed and len(keep) != len(insts):
            del blk.instructions[:]
            for inst in keep:
                blk.instructions.append(inst)
    return removed


def build_kernel(rv_vals):
    """Build the single-core Bass module (SPMD: same program on all 8 cores).

    rv_vals: the 5 rv scalars, baked as immediates into the weight-gen ops.
    """
    nc = Bacc()
    x_p = nc.declare_dram_parameter("x", [BPC, C, H, W], F16, isOutput=False)
    m_p = nc.declare_dram_parameter("M", [C, C, KS, KS], F32, isOutput=False)
    # Z rides as fp16: its contribution to w is scaled by rv (~4.5e-3), so
    # fp16 rounding perturbs w by ~3e-7 against w's sigma of 0.03 — ~0.3
    # expected weight-sign flips across all 147K weights (deterministic for
    # the fixed inputs; measured rel err stays ~2e-4).  M stays fp32 (it IS
    # w's magnitude; fp16 M would flip ~100x more signs).
    z_p = nc.declare_dram_parameter("Z", [K, C, C, KS, KS], F16, isOutput=False)
    a_p = nc.declare_dram_parameter("Alpha", [C, 1, 1], F32, isOutput=False)
    rv_p = nc.declare_dram_parameter("rv", [1, K], F32, isOutput=False)
    # The output stays in the conv's padded-row layout (7 chunks x 8 rows x
    # 58 cols incl. 2 garbage cols): evictions and output DMAs are then flat
    # 2D copies (the 3D strided PSUM read costs ScalarE ~1.6x, and a strided
    # output DMA is descriptor-bound and catastrophically slow).  The host
    # strips the garbage columns during the gather.  +3.5% output bytes.
    out_p = nc.declare_dram_parameter(
        "out", [BPC, C, N_CHUNKS * FREE_R], F16, isOutput=True
    )

    NW = C * KS * KS  # 1152 weight elements per out-channel row
    HALF = (H // 2) * W  # first-half image elements (28 rows)
    HGEN = NW // 2  # weight-gen half-column split (pipelines vs the z DMAs)

    with tile.TileContext(nc) as tc, ExitStack() as ctx:
        const = ctx.enter_context(tc.tile_pool(name="const", bufs=1))
        wg = ctx.enter_context(tc.tile_pool(name="wg", bufs=1))
        zpool = ctx.enter_context(tc.tile_pool(name="zpool", bufs=1))
        xin = ctx.enter_context(tc.tile_pool(name="xin", bufs=BPC))
        pad = ctx.enter_context(tc.tile_pool(name="pad", bufs=1))
        opool = ctx.enter_context(tc.tile_pool(name="opool", bufs=3))
        ps = ctx.enter_context(tc.tile_pool(name="ps", bufs=1, space="PSUM"))

        # ---- constants ----
        # Anti-diagonal permutation: transpose against it yields the transposed
        # tap with REVERSED out-channel columns, which is exactly the column
        # order DoubleRowSwInterleave's weight layout wants.
        # The tile name doubles as a NEFF-cache marker for the ldw-opt flag
        # (the cache keys on BIR content, not compiler flags).
        identity = const.tile([C, C], BF16, name=f"identity_ldw{int(LDW_OPT)}")
        nc.gpsimd.memset(identity[:], 0.0)
        nc.gpsimd.affine_select(
            out=identity[:],
            in_=identity[:],
            compare_op=mybir.AluOpType.not_equal,
            fill=1.0,
            base=-(C - 1),
            pattern=[[1, C]],
            channel_multiplier=1,
        )
        # Alpha/rv ride the ACT HWDGE ring so the SP ring's first slots
        # belong to x0/Z.
        alpha_sb = const.tile([C, 1], F32)
        nc.scalar.dma_start(alpha_sb[:], a_p[:].rearrange("c a b -> c (a b)"))
        rv_sb = const.tile([1, K], F32)
        nc.scalar.dma_start(rv_sb[:], rv_p[:])

        x_ap = x_p[:]
        o_ap = out_p[:]

        # The HWDGE SP ring drains FIFO, so this issue order is the wire
        # order.  The weight-gen STT chain paces to z_k arrivals (one STT per
        # landing), so Z goes out early, interleaved with image 0's halves
        # (whose signs must finish before the conv, ~16us in).  All remaining
        # images are issued up front (xin has BPC bufs, nothing recycles, the
        # ring never starves).
        # Wire order tuned so each landing's trailing compute hides inside
        # the remaining wire: z0-z3 (the DVE chain starts on z0 and is
        # throughput-bound after that), x0's halves (ACT signs run during the
        # z4/m wire), then z4, then M (folded into the chain LAST), then the
        # remaining images.  Whole-tensor z DMAs: half-DMAs doubled the
        # ~650ns-per-issue load on the SP sequencer for no wire gain.
        x_sbs = []
        x_sbs.append(xin.tile([C, H * W], F16, name="x_sb0", tag="x_sb"))
        z_sbs = []
        for k in range(K):
            z_sbs.append(zpool.tile([C, NW], F16, name=f"z{k}", tag=f"z{k}"))
        for k in range(K - 1):
            nc.sync.dma_start(
                z_sbs[k][:], z_p[k].rearrange("o i a b -> o (i a b)")
            )
        nc.sync.dma_start(
            x_sbs[0][:, 0:HALF], x_ap[0].rearrange("c h w -> c (h w)")[:, 0:HALF]
        )
        nc.sync.dma_start(
            x_sbs[0][:, HALF:], x_ap[0].rearrange("c h w -> c (h w)")[:, HALF:]
        )
        nc.sync.dma_start(
            z_sbs[K - 1][:], z_p[K - 1].rearrange("o i a b -> o (i a b)")
        )
        m_sb = wg.tile([C, NW], F32)
        nc.sync.dma_start(m_sb[:], m_p[:].rearrange("o i a b -> o (i a b)"))
        for i in range(1, BPC):
            x_sbs.append(xin.tile([C, H * W], F16, name=f"x_sb{i}", tag="x_sb"))
            nc.sync.dma_start(
                x_sbs[i][:], x_ap[i].rearrange("c h w -> c (h w)")
            )

        # ---- padded sign buffers: 3 physical buffers, borders zeroed ONCE.
        # Every image only writes the interior, so the zero border persists
        # across reuses.
        ba_bufs = []
        for b in range(3):
            ba = pad.tile([C, PH * PW + 2], F8, name=f"ba{b}", tag=f"ba{b}")
            ba_r = ba[:, 0 : PH * PW].rearrange("c (h w) -> c h w", w=PW)
            nc.gpsimd.memset(ba[:, 0:PW], 0.0)
            nc.gpsimd.memset(ba[:, (PH - 1) * PW : PH * PW + 2], 0.0)
            nc.gpsimd.memset(ba_r[:, 1 : H + 1, 0:1], 0.0)
            nc.gpsimd.memset(ba_r[:, 1 : H + 1, W + 1 : PW], 0.0)
            ba_bufs.append(ba)

        # ---- weight generation: w = (sum_k rv_k Z_k) + M  (DVE; Pool lacks
        # the TensorScalarPtr op).  The rv@Z partial sums are tiny (~3e-4,
        # rv-scaled), so they accumulate in fp16 — which makes every chain
        # operand 2-byte and unlocks DVE's 2x_1p mode, halving the 5 chain
        # ops.  M (w's actual magnitude) folds in LAST at fp32, so its DMA
        # can also trail the (critical) Z wire.
        w16 = wg.tile([C, NW], F16)
        w_sb = wg.tile([C, NW], F32)
        nc.vector.tensor_scalar_mul(w16[:], z_sbs[0][:], float(rv_vals[0]))
        for k in range(1, K):
            nc.vector.scalar_tensor_tensor(
                w16[:],
                z_sbs[k][:],
                float(rv_vals[k]),
                w16[:],
                mybir.AluOpType.mult,
                mybir.AluOpType.add,
            )
        nc.vector.tensor_add(w_sb[:], w16[:], m_sb[:])
        bw_sb = wg.tile([C, NW], BF16)

        def psum_tile(ch, shape, dtype, name):
            # The one spare bank double-buffers pt5: chunk 5's eviction is
            # ScalarE's third (done ~T+3.9us after a pass-4 sweep starting at
            # T) while image i+1's pass-0 wants the bank at ~T+2.4us — the
            # worst boundary stall.  Chunk 0's eviction (DVE's first, T+0.9)
            # always beats its T+1.4 demand, so pt0 needs no double buffer.
            return ps.tile(
                shape, dtype, name=name, tag=f"pt{ch}", bufs=(2 if ch == 5 else 1)
            )

        def sign_image(i, halves=False):
            """Binarize image i's fp16 pixels into its ba buffer interior."""
            ba = ba_bufs[i % 3]
            ba_r = ba[:, 0 : PH * PW].rearrange("c (h w) -> c h w", w=PW)
            x_r = x_sbs[i][:].rearrange("c (h w) -> c h w", w=W)
            if halves:
                nc.scalar.sign(ba_r[:, 1 : H // 2 + 1, 1 : W + 1], x_r[:, : H // 2])
                nc.scalar.sign(ba_r[:, H // 2 + 1 : H + 1, 1 : W + 1], x_r[:, H // 2 :])
            else:
                nc.scalar.sign(ba_r[:, 1 : H + 1, 1 : W + 1], x_r)
            return ba

        # Image 0's sign runs as soon as its half-DMAs land, before the
        # weight sign (which waits on the Z chain) enters the ACT queue.
        sign_image(0, halves=True)
        nc.scalar.sign(bw_sb[:], w_sb[:])
        sign_image(1)

        # Transpose each tap's [oc, ic] into [ic, oc-reversed] (via the
        # anti-diagonal permutation), then interleave tap pairs column-wise as
        # fp8e4 (+-1 exact): the DoubleRowSwInterleave weight layout.  The
        # transposes borrow the conv's PSUM banks (idle during the prologue).
        # Chain every PE matmul (transposes included) in emission order with
        # ordering-only deps: the tile scheduler otherwise interleaves the
        # weight passes, breaking the same-weights runs the LDWEIGHTS
        # excision needs.
        pe_chain = [None]

        from concourse.instruction_name_ordered_set import (
            InstructionNameOrderedSet,
        )

        def chain_pe(bi):
            raw = bi.ins
            if pe_chain[0] is not None:
                s = InstructionNameOrderedSet()
                s.add(pe_chain[0])
                raw.add_nosync_dependencies_from(s)
            pe_chain[0] = raw.name

        wt = const.tile([C, 5, 2 * C], F8)
        nc.vector.memset(wt[:, 4, :], 0.0)
        bw_r = bw_sb[:].rearrange("o (i j) -> o i j", j=KS * KS)
        # Transposes j7/j8 reuse banks pt5/pt6 — the banks image 0's first
        # pass touches LAST — so their trailing wt-copies never stall it
        # (j%7 would park the two latest copies on pt0/pt1, the banks pass 0
        # needs FIRST, costing ~1.5us).
        TP_BANK = [0, 1, 2, 3, 4, 5, 6, 5, 6]
        for j in range(KS * KS):
            tp = psum_tile(TP_BANK[j], [C, C], BF16, f"tp{j}")
            chain_pe(nc.tensor.transpose(tp[:], bw_r[:, :, j], identity[:]))
            pair, slot = divmod(j, 2)
            wt_h = wt[:].tensor
            dst = bass.AP(wt_h, pair * 2 * C + slot, [[5 * 2 * C, C], [2, C]])
            nc.vector.tensor_copy(dst, tp[:])
        # rv reaches the kernel as baked immediates; touch the tensor so the
        # bound input isn't dead.
        nc.vector.tensor_copy(w_sb[0:1, 0:K], rv_sb[0:1, :])

        def tap_off(r0, j):
            # flat offset of (out-row r0, tap j)'s top-left read in the padded image
            if j == KS * KS:  # zero tap: alias tap 8's window (weights are 0)
                j = KS * KS - 1
            return (r0 + j // KS) * PW + (j % KS)

        # Eviction engine per chunk: GpSimd has no PSUM port, so split
        # DVE/ScalarE; ScalarE also carries the signs.
        # DVE takes {0,2,4,5}: trace decode showed the per-boundary stalls
        # are pass-0's ch4/ch6 matmuls waiting the DVE semaphore — DVE's
        # serialized queue delivered e6 at T+2.96 vs its T+2.6 demand.  ch6
        # on ScalarE arrives at T+2.42 (3rd in its queue), and ch5, now
        # DVE's last delivery, rides the double-buffered pt5 bank where lag
        # is free.
        EVICT_DVE = (0, 2, 4, 5)

        def conv_image(i, ba):
            """5 weight passes x 7 chunk matmuls into 7 PSUM banks, then
            alpha-scaled eviction to fp16 and the output DMA."""
            pts = [
                psum_tile(ch, [C, 512], F32, f"pt{ch}_{i}") for ch in range(N_CHUNKS)
            ]
            # Pass 0's sweep order matches each bank's eviction-delivery
            # order: ch5 (double-buffered, lag-free) absorbs the T+2.2 slot
            # so ch4 (DVE's 3rd delivery, T+2.27) and ch6 (ScalarE's 3rd,
            # T+2.42) each gain ~200ns of slack over the ~250ns semaphore
            # latency, removing the last per-boundary stall.
            P0_ORDER = (0, 1, 2, 3, 5, 4, 6)
            for p in range(5):
                for ch in P0_ORDER if p == 0 else range(N_CHUNKS):
                    r0 = ch * CHUNK_ROWS
                    o0 = tap_off(r0, 2 * p)
                    o1 = tap_off(r0, 2 * p + 1)
                    rhs = bass.AP(
                        ba[:].tensor,
                        o0,
                        [[PH * PW + 2, C], [o1 - o0, 2], [1, FREE_R]],
                    )
                    mi = nc.tensor.matmul(
                        pts[ch][:, 0:FREE_R],
                        wt[:, p, :],
                        rhs,
                        start=(p == 0),
                        stop=(p == 4),
                        perf_mode=mybir.MatmulPerfMode.DoubleRowSwInterleave,
                    )
                    chain_pe(mi)
            o_sb = opool.tile(
                [C, N_CHUNKS * FREE_R], F16, name=f"o_sb{i}", tag="o_sb"
            )
            for ch in range(N_CHUNKS):
                src = pts[ch][:, 0:FREE_R]
                dst = o_sb[:, ch * FREE_R : (ch + 1) * FREE_R]
                if ch in EVICT_DVE:
                    nc.vector.tensor_scalar_mul(dst, src, alpha_sb[:, 0:1])
                else:
                    nc.scalar.mul(dst, src, alpha_sb[:, 0:1])
            return o_sb

        # Software-pipelined image loop.  Signs for images 0/1 were emitted
        # above; each iteration's sign(i+2) is emitted AFTER image i's
        # ScalarE evictions so it doesn't delay them in the ACT queue.
        for i in range(BPC):
            o_sb = conv_image(i, ba_bufs[i % 3])
            if i + 2 < BPC:
                sign_image(i + 2)
            o_hbm = o_ap[i]
            if i < 5:
                # Early outputs ride SWDGE (GpSimd) so they never head-of-line
                # block input loads on the FIFO HWDGE SP ring.
                nc.gpsimd.dma_start(o_hbm, o_sb[:])
            elif i < BPC - 1:
                # All input issues are done by ~20us; the SP ring is idle.
                nc.sync.dma_start(o_hbm, o_sb[:])
            else:
                # Final image drains in 2-chunk pieces, alternating the two
                # idle HWDGE rings, each issued as soon as its chunks evict.
                F2 = 2 * FREE_R
                nc.sync.dma_start(o_hbm[:, 0:F2], o_sb[:, 0:F2])
                nc.scalar.dma_start(o_hbm[:, F2 : 2 * F2], o_sb[:, F2 : 2 * F2])
                nc.sync.dma_start(
                    o_hbm[:, 2 * F2 : 3 * F2], o_sb[:, 2 * F2 : 3 * F2]
                )
                nc.scalar.dma_start(o_hbm[:, 3 * F2 :], o_sb[:, 3 * F2 :])

    if LDW_ELIDE:
        _excise_redundant_ldweights(nc)
    nc.finalize()
    return nc


_CACHE = {}


def _get_nc(rv):
    key = rv.tobytes()
    if key not in _CACHE:
        _CACHE[key] = build_kernel(np.asarray(rv, np.float32).reshape(-1))
    return _CACHE[key]


def _run(inputs, trace=False):
    x = np.ascontiguousarray(np.asarray(inputs["x"]), dtype=np.float16)
    M = np.ascontiguousarray(np.asarray(inputs["M"], np.float32))
    Z = np.ascontiguousarray(np.asarray(inputs["Z"]), dtype=np.float16)
    Alpha = np.ascontiguousarray(np.asarray(inputs["Alpha"], np.float32))
    rv = np.ascontiguousarray(np.asarray(inputs["rv"], np.float32))
    nc = _get_nc(rv)
    in_maps = [
        {"x": x[c * BPC : (c + 1) * BPC], "M": M, "Z": Z, "Alpha": Alpha, "rv": rv}
        for c in range(N_CORES)
    ]
    res = run_bass_kernel_spmd(nc, in_maps, list(range(N_CORES)), trace=trace)
    out = np.concatenate([res.results[c]["out"] for c in range(N_CORES)], axis=0)
    return strip_pad(out), res


def strip_pad(out):
    """[B?, C, 7*464] padded-row device output -> [B?, C, H, W] fp32."""
    out = np.asarray(out)
    b = out.shape[0]
    out = out.reshape(b, C, N_CHUNKS * CHUNK_ROWS, PW)[:, :, :, 0:W]
    return np.ascontiguousarray(out, dtype=np.float32)


def kernel(**inputs):
    out, _ = _run(inputs, trace=False)
    return out


def kernel_traced(**inputs):
    out, res = _run(inputs, trace=True)
    return out, res

